# revision 1
# baseline (speedup 1.0000x reference)
"""Trainium2 Bass kernel for nn_KnowledgeFusion.

Math (b=8, H=W=32, d=o=256, n_obj=15, n=16 with appended mean-emb):
  embs_aug = concat([embs, mean(embs)])                  [b,16,256]
  mask     = rasterized boxes (rounded to PATCH_SIZE=2)  [b,16,1024] in {0,1}
  proj     = patches @ Wp                                [b,1024,256]
  inj      = embs_aug @ We                               [b,16,256]
  s[hw]    = sum_n mask[n,hw]   (>=1: image box row)
  out      = proj + (mask^T @ inj) / s[:,None]           [b,1024,256]

(The reference's (proj + m*inj) masked-mean collapses to this because
mask^2 == mask.)

Sharding: data-parallel over batch; core c computes batch c (Wp/We
replicated). Computed in the transposed orientation outT[o, hw] so Wp
(resp. inj) is the stationary matmul operand and the 1024-pixel axis
streams at N=512 per matmul:

  outT[o,hw] = Wp^T @ patchesT  +  inj^T @ maskN        maskN = mask/s

All matmuls run as float32r (single-pass fp32, ~4x the fp32 rate, fp32
PSUM accumulation). The 1/s normalization is folded into the mask so
proj and the injection accumulate in the same PSUM bank; 1/s itself is
computed exactly without any slow reciprocal: s is an integer in 1..16,
so broadcast s over 16 partitions (all-ones matmul), take the indicator
ind[n,hw] = (s == n+1), and matmul against weights 1/(n+1).

Inputs arrive via 3 DMAs (tiny loc first -- the mask chain is the
latency pole -- then a weights blob, then patchesT) because each
dma_start costs ~0.6us of sequencer dispatch; outputs leave via 2.
"""

import sys

sys.path.insert(0, "/opt/trn_rl_repo")

import numpy as np

import concourse.bass as bass
import concourse.bacc as bacc
import concourse.mybir as mybir
from concourse import tile
from concourse import bass_utils
from concourse.alu_op_type import AluOpType

B, H, W, D = 8, 32, 32, 256
NOBJ, N = 15, 16
HW = H * W
O = 256
FP = mybir.dt.float32
FR = mybir.dt.float32r
I32 = mybir.dt.int32
AF = mybir.ActivationFunctionType
AX = mybir.AxisListType

# weights blob layout (columns): Wp0 Wp1 We0 We1 eT0 eT1 (each eT chunk
# has 15 real columns + 1 spare for the on-device mean)
WB = 2 * O + 2 * O + 2 * N  # 1056


def _bcast(ap, free_dims):
    """AP with explicit free-dim [step, count] pairs (step 0 = broadcast)."""
    return bass.AP(ap.tensor, ap.offset, ap.ap[:1] + free_dims)


def build_nc(debug: bool = False):
    nc = bacc.Bacc("TRN2", target_bir_lowering=False, debug=debug, num_devices=B)

    loc = nc.dram_tensor("loc", [N, 4], I32, kind="ExternalInput")
    wb = nc.dram_tensor("wb", [128, WB], FR, kind="ExternalInput")
    pT = nc.dram_tensor("pT", [128, 2 * HW], FR, kind="ExternalInput")
    outT = nc.dram_tensor("outT", [O, HW], FP, kind="ExternalOutput")

    with tile.TileContext(nc) as tc:
        with (
            nc.allow_low_precision(reason="fp32r matmuls, fp32 PSUM accumulation"),
            tc.tile_pool(name="big", bufs=1) as big,
            tc.tile_pool(name="small", bufs=1) as small,
            tc.tile_pool(name="outp", bufs=2) as outp,
            tc.tile_pool(name="psT", bufs=4, space=bass.MemorySpace.PSUM) as psT,
            tc.tile_pool(name="pstmp", bufs=2, space=bass.MemorySpace.PSUM) as pstmp,
        ):
            # ---- loads: loc first (mask chain is the long pole)
            loc_sb = small.tile([N, 4], I32)
            nc.sync.dma_start(loc_sb[:], loc[:])
            wb_sb = big.tile([128, WB], FR)
            nc.sync.dma_start(wb_sb[:], wb[:])
            pT_sb = big.tile([128, 2 * HW], FR)
            nc.gpsimd.dma_start(pT_sb[:, 0:HW], pT[:, 0:HW])
            nc.sync.dma_start(pT_sb[:, HW : 2 * HW], pT[:, HW : 2 * HW])

            Wp_sb = [wb_sb[:, O * k : O * (k + 1)] for k in range(2)]
            We_sb = [wb_sb[:, 2 * O + O * k : 2 * O + O * (k + 1)] for k in range(2)]
            eT_sb = [wb_sb[:, 4 * O + N * k : 4 * O + N * (k + 1)] for k in range(2)]

            # mean embedding into the spare 16th column of each eT chunk
            for k in range(2):
                nc.vector.tensor_reduce(
                    eT_sb[k][:, NOBJ : NOBJ + 1], eT_sb[k][:, 0:NOBJ], AX.X, AluOpType.add
                )
                nc.vector.tensor_scalar_mul(
                    eT_sb[k][:, NOBJ : NOBJ + 1], eT_sb[k][:, NOBJ : NOBJ + 1], 1.0 / NOBJ
                )

            # ---- inj = embs_aug @ We -> [16, 256]
            psumI = pstmp.tile([N, O], FP, tag="pstmp")
            nc.tensor.matmul(psumI[:], eT_sb[0][:], We_sb[0][:], start=True, stop=False)
            nc.tensor.matmul(psumI[:], eT_sb[1][:], We_sb[1][:], start=False, stop=True)
            inj_sb = small.tile([N, O], FR)
            nc.scalar.activation(inj_sb[:], psumI[:], AF.Copy)

            # ---- boxes: round starts down / ends up to multiples of 2
            locm = small.tile([N, 4], I32)
            nc.vector.tensor_scalar(locm[:], loc_sb[:], 1, None, op0=AluOpType.bitwise_and)
            boxes_i = small.tile([N, 4], I32)
            nc.vector.tensor_tensor(boxes_i[:], loc_sb[:], locm[:], op=AluOpType.subtract)
            nc.vector.tensor_scalar_add(boxes_i[:, 2:4], boxes_i[:, 2:4], 2)
            boxes_f = small.tile([N, 4], FP)
            nc.vector.tensor_copy(boxes_f[:], boxes_i[:])

            # ---- row/col interval masks [16, 32]
            grid_i = small.tile([N, 32], I32)
            nc.gpsimd.iota(grid_i[:], pattern=[[1, 32]], base=0, channel_multiplier=0)
            grid_f = small.tile([N, 32], FP)
            nc.vector.tensor_copy(grid_f[:], grid_i[:])

            rowm = small.tile([N, 32], FP)
            colm = small.tile([N, 32], FP)
            tmp = small.tile([N, 32], FP, tag="cmp_tmp")
            nc.vector.tensor_scalar(tmp[:], grid_f[:], boxes_f[:, 2:3], None, op0=AluOpType.is_lt)
            nc.vector.scalar_tensor_tensor(
                rowm[:], grid_f[:], boxes_f[:, 0:1], tmp[:], op0=AluOpType.is_ge, op1=AluOpType.mult
            )
            tmp2 = small.tile([N, 32], FP, tag="cmp_tmp2")
            nc.vector.tensor_scalar(tmp2[:], grid_f[:], boxes_f[:, 3:4], None, op0=AluOpType.is_lt)
            nc.vector.scalar_tensor_tensor(
                colm[:], grid_f[:], boxes_f[:, 1:2], tmp2[:], op0=AluOpType.is_ge, op1=AluOpType.mult
            )

            # ---- mask [16, 1024] via one broadcast outer-product multiply
            mask_sb = small.tile([N, HW], FR)
            nc.vector.tensor_tensor(
                _bcast(mask_sb[:], [[W, H], [1, W]]),
                _bcast(rowm[:], [[1, H], [0, W]]),
                _bcast(colm[:], [[0, H], [1, W]]),
                op=AluOpType.mult,
            )

            # ---- 1/s exactly, no reciprocal over hw: s integer in 1..16
            ones1c = small.tile([N, 1], FP)
            nc.vector.memset(ones1c[:], 1.0)
            ones16 = small.tile([N, N], FR)
            nc.vector.tensor_copy(ones16[:], _bcast(ones1c[:], [[0, N]]))
            idx_i = small.tile([N, 1], I32)
            nc.gpsimd.iota(idx_i[:], pattern=[[1, 1]], base=1, channel_multiplier=1)
            kvec = small.tile([N, 1], FP)
            nc.vector.tensor_copy(kvec[:], idx_i[:])
            wn = small.tile([N, 1], FP)
            nc.vector.reciprocal(wn[:], kvec[:])
            w16 = small.tile([N, N], FR)
            nc.vector.tensor_copy(w16[:], _bcast(wn[:], [[0, N]]))

            ind_sb = small.tile([N, HW], FR)
            psumS = [pstmp.tile([N, 512], FP, tag="pstmp", name=f"psS{h}") for h in range(2)]
            for h in range(2):
                nc.tensor.matmul(
                    psumS[h][:], ones16[:], mask_sb[:, 512 * h : 512 * (h + 1)],
                    start=True, stop=True,
                )
                nc.vector.tensor_scalar(
                    ind_sb[:, 512 * h : 512 * (h + 1)], psumS[h][:], kvec[:, 0:1], None,
                    op0=AluOpType.is_equal,
                )

            recB_sb = small.tile([N, HW], FP)
            psumR = [pstmp.tile([N, 512], FP, tag="pstmp", name=f"psR{h}") for h in range(2)]
            for h in range(2):
                nc.tensor.matmul(
                    psumR[h][:], w16[:], ind_sb[:, 512 * h : 512 * (h + 1)],
                    start=True, stop=True,
                )
                nc.scalar.activation(recB_sb[:, 512 * h : 512 * (h + 1)], psumR[h][:], AF.Copy)

            # ---- maskN = mask * recB  (the /s folded into the mask)
            maskN_sb = small.tile([N, HW], FR)
            nc.vector.tensor_tensor(maskN_sb[:], mask_sb[:], recB_sb[:], op=AluOpType.mult)

            # ---- main: outT[oc*128:, :] = Wp^T @ pT + inj^T @ maskN
            for oc in range(2):
                o0 = 128 * oc
                o_sb = outp.tile([128, HW], FP, tag="osb")
                for hc in range(2):
                    h0 = 512 * hc
                    psum = psT.tile([128, 512], FP, tag="psT")
                    nc.tensor.matmul(
                        psum[:], Wp_sb[0][:, o0 : o0 + 128],
                        pT_sb[:, h0 : h0 + 512],
                        start=True, stop=False,
                    )
                    nc.tensor.matmul(
                        psum[:], Wp_sb[1][:, o0 : o0 + 128],
                        pT_sb[:, HW + h0 : HW + h0 + 512],
                        start=False, stop=False,
                    )
                    nc.tensor.matmul(
                        psum[:], inj_sb[:, o0 : o0 + 128], maskN_sb[:, h0 : h0 + 512],
                        start=False, stop=True,
                    )
                    if hc == 0:
                        nc.vector.tensor_copy(o_sb[:, h0 : h0 + 512], psum[:])
                    else:
                        nc.scalar.activation(o_sb[:, h0 : h0 + 512], psum[:], AF.Copy)
                eng = nc.sync if oc == 0 else nc.gpsimd
                eng.dma_start(outT[o0 : o0 + 128, :], o_sb[:])

    nc.compile()
    return nc


def make_in_maps(inputs):
    patches = np.asarray(inputs["patches"], dtype=np.float32)
    embs = np.asarray(inputs["embs"], dtype=np.float32)
    locations = np.asarray(inputs["locations"], dtype=np.int32)
    Wp = np.asarray(inputs["Wp"], dtype=np.float32)
    We = np.asarray(inputs["We"], dtype=np.float32)
    img_box = np.array([[0, 0, H, W]], dtype=np.int32)
    wb_common = np.zeros((128, WB), dtype=np.float32)
    wb_common[:, 0:O] = Wp[0:128]
    wb_common[:, O : 2 * O] = Wp[128:256]
    wb_common[:, 2 * O : 3 * O] = We[0:128]
    wb_common[:, 3 * O : 4 * O] = We[128:256]
    in_maps = []
    for b in range(B):
        eTb = embs[b].T  # [256, 15]
        wbb = wb_common.copy()
        wbb[:, 4 * O : 4 * O + NOBJ] = eTb[0:128]
        wbb[:, 4 * O + N : 4 * O + N + NOBJ] = eTb[128:256]
        pTb = patches[b].reshape(HW, D).T  # [256, 1024]
        pT2 = np.concatenate([pTb[0:128], pTb[128:256]], axis=1)  # [128, 2048]
        in_maps.append(
            {
                "loc": np.ascontiguousarray(np.concatenate([locations[b], img_box], 0)),
                "wb": wbb,
                "pT": np.ascontiguousarray(pT2),
            }
        )
    return in_maps


_NC = None


def _get_nc():
    global _NC
    if _NC is None:
        _NC = build_nc(debug=False)
    return _NC


def run(inputs, trace: bool = False, **kwargs):
    nc = _get_nc()
    res = bass_utils.run_bass_kernel_spmd(
        nc, make_in_maps(inputs), core_ids=list(range(B)), trace=trace, **kwargs
    )
    full = np.stack([res.results[b]["outT"].T for b in range(B)], axis=0)
    return np.ascontiguousarray(full).astype(np.float32), res


def kernel(**inputs) -> np.ndarray:
    full, _ = run(inputs, trace=False)
    return full



# revision 2
# speedup vs baseline: 1.3709x; 1.3709x over previous
"""Trainium2 Bass kernel for nn_KnowledgeFusion (bf16 pipeline).

Math (b=8, H=W=32, d=o=256, n_obj=15, n=16 with appended mean-emb):
  embs_aug = concat([embs, mean(embs)])                  [b,16,256]
  mask     = rasterized boxes (rounded to PATCH_SIZE=2)  [b,16,1024] in {0,1}
  proj     = patches @ Wp                                [b,1024,256]
  inj      = embs_aug @ We                               [b,16,256]
  s[hw]    = sum_n mask[n,hw]   (>=1: image box row)
  out      = proj + (mask^T @ inj) / s[:,None]           [b,1024,256]

Sharding: data-parallel over batch; core c computes batch c. Computed
transposed, outT[o, hw] = Wp^T @ patchesT + inj^T @ maskN, maskN =
mask/s. All tensor data is bf16 (fp32 PSUM accumulation); measured
end-to-end rel err ~4e-3 vs the fp32 reference (gate 2e-2).

1/s is exact: s is an integer in 1..16 broadcast to 16 partitions by an
all-ones matmul; indicator ind[k,hw] = (s == k+1); recB = w16 @ ind with
w16 rows 1/(k+1).

Schedule notes (vs the 34us fp32 version this replaces):
 - bf16 halves DMA traffic and enables fast weight load.
 - hdr (loc + iota grid + 1..16 col constants) rides one tiny early DMA
   on the scalar queue; wb on sync; pT halves on gpsimd. Outputs go out
   per o-chunk on sync/scalar as soon as each is evacuated.
 - a burst of dummy matmuls on a memset tile warms the PE clock (HAM)
   during the input DMA window.
 - the s-chain (ones-mm -> is_eq -> w16-mm -> recB -> maskN) only gates
   the trailing 4 injection matmuls; the 8 projection matmuls run as
   soon as pT lands.
"""

import sys

sys.path.insert(0, "/opt/trn_rl_repo")

import numpy as np
import ml_dtypes

import concourse.bass as bass
import concourse.bacc as bacc
import concourse.mybir as mybir
from concourse import tile
from concourse import bass_utils
from concourse.alu_op_type import AluOpType

B, H, W, D = 8, 32, 32, 256
NOBJ, N = 15, 16
HW = H * W
O = 256
FP = mybir.dt.float32
BF = mybir.dt.bfloat16
I32 = mybir.dt.int32
AF = mybir.ActivationFunctionType
AX = mybir.AxisListType

# wb blob columns (bf16): Wp0 Wp1 We0 We1 eT0 eT1 (eT chunks have 15
# real columns + 1 spare for the on-device mean)
WB = 2 * O + 2 * O + 2 * N  # 1056
# hdr columns (i32): loc[4] grid[32] kcol[1]
HDR = 4 + 32 + 1

N_WARM = 10  # dummy matmuls to warm the PE clock during input DMA


def _bcast(ap, free_dims):
    """AP with explicit free-dim [step, count] pairs (step 0 = broadcast)."""
    return bass.AP(ap.tensor, ap.offset, ap.ap[:1] + free_dims)


def build_nc(debug: bool = False):
    nc = bacc.Bacc("TRN2", target_bir_lowering=False, debug=debug, num_devices=B)

    hdr = nc.dram_tensor("hdr", [N, HDR], I32, kind="ExternalInput")
    wb = nc.dram_tensor("wb", [128, WB], BF, kind="ExternalInput")
    pT = nc.dram_tensor("pT", [128, 2 * HW], BF, kind="ExternalInput")
    outT = nc.dram_tensor("outT", [O, HW], BF, kind="ExternalOutput")

    with tile.TileContext(nc) as tc:
        with (
            nc.allow_low_precision(reason="bf16 matmuls, fp32 PSUM accumulation"),
            tc.tile_pool(name="big", bufs=1) as big,
            tc.tile_pool(name="small", bufs=1) as small,
            tc.tile_pool(name="outp", bufs=2) as outp,
            tc.tile_pool(name="psM", bufs=4, space=bass.MemorySpace.PSUM) as psM,
            tc.tile_pool(name="psS", bufs=2, space=bass.MemorySpace.PSUM) as psS,
            tc.tile_pool(name="psI", bufs=1, space=bass.MemorySpace.PSUM) as psI,
            tc.tile_pool(name="psW", bufs=1, space=bass.MemorySpace.PSUM) as psW,
        ):
            # ---- input DMAs: hdr tiny+early (scalar q), wb (sync q),
            # pT halves (gpsimd q)
            hdr_sb = small.tile([N, HDR], I32)
            nc.scalar.dma_start(hdr_sb[:], hdr[:])
            wb_sb = big.tile([128, WB], BF)
            nc.sync.dma_start(wb_sb[:], wb[:])
            pT_sb = big.tile([128, 2 * HW], BF)
            nc.gpsimd.dma_start(pT_sb[:, 0:HW], pT[:, 0:HW])
            nc.gpsimd.dma_start(pT_sb[:, HW : 2 * HW], pT[:, HW : 2 * HW])

            Wp_sb = [wb_sb[:, O * k : O * (k + 1)] for k in range(2)]
            We_sb = [wb_sb[:, 2 * O + O * k : 2 * O + O * (k + 1)] for k in range(2)]
            eT_sb = [wb_sb[:, 4 * O + N * k : 4 * O + N * (k + 1)] for k in range(2)]

            # ---- PE warm-up: dummy matmuls on a memset tile
            wtile = small.tile([128, 256], BF, tag="wtile")
            nc.vector.memset(wtile[:], 0.0)
            warm_ps = psW.tile([128, 256], FP, tag="warm")
            for _ in range(N_WARM):
                nc.tensor.matmul(
                    warm_ps[:], wtile[:, 0:128], wtile[:], start=True, stop=True
                )

            # ---- constants
            ones16 = small.tile([N, N], BF)
            nc.vector.memset(ones16[:], 1.0)
            kvec = small.tile([N, 1], FP)
            nc.vector.tensor_copy(kvec[:], hdr_sb[:, 36:37])
            wcol = small.tile([N, 1], FP)
            nc.vector.reciprocal(wcol[:], kvec[:])
            w16 = small.tile([N, N], BF)
            nc.vector.tensor_copy(w16[:], _bcast(wcol[:], [[0, N]]))
            grid_f = small.tile([N, 32], FP)
            nc.vector.tensor_copy(grid_f[:], hdr_sb[:, 4:36])

            # ---- boxes: round starts down / ends up to multiples of 2
            loc_sb = hdr_sb[:, 0:4]
            locm = small.tile([N, 4], I32)
            nc.vector.tensor_scalar(locm[:], loc_sb, 1, None, op0=AluOpType.bitwise_and)
            boxes_i = small.tile([N, 4], I32)
            nc.vector.tensor_tensor(boxes_i[:], loc_sb, locm[:], op=AluOpType.subtract)
            nc.vector.tensor_scalar_add(boxes_i[:, 2:4], boxes_i[:, 2:4], 2)
            boxes_f = small.tile([N, 4], FP)
            nc.vector.tensor_copy(boxes_f[:], boxes_i[:])

            # ---- row/col interval masks [16, 32] fp32
            rowm = small.tile([N, 32], FP)
            colm = small.tile([N, 32], FP)
            tmp = small.tile([N, 32], FP, tag="cmp_tmp")
            nc.vector.tensor_scalar(tmp[:], grid_f[:], boxes_f[:, 2:3], None, op0=AluOpType.is_lt)
            nc.vector.scalar_tensor_tensor(
                rowm[:], grid_f[:], boxes_f[:, 0:1], tmp[:], op0=AluOpType.is_ge, op1=AluOpType.mult
            )
            tmp2 = small.tile([N, 32], FP, tag="cmp_tmp2")
            nc.vector.tensor_scalar(tmp2[:], grid_f[:], boxes_f[:, 3:4], None, op0=AluOpType.is_lt)
            nc.vector.scalar_tensor_tensor(
                colm[:], grid_f[:], boxes_f[:, 1:2], tmp2[:], op0=AluOpType.is_ge, op1=AluOpType.mult
            )

            # ---- mask [16, 1024] bf16 via broadcast outer product, in
            # halves so the s-chain can start on half 0 early
            mask_sb = small.tile([N, HW], BF, tag="mask")
            for h in range(2):
                nc.vector.tensor_tensor(
                    _bcast(mask_sb[:, 512 * h : 512 * (h + 1)], [[W, 16], [1, W]]),
                    _bcast(rowm[:, 16 * h : 16 * h + 16], [[1, 16], [0, W]]),
                    _bcast(colm[:], [[0, 16], [1, W]]),
                    op=AluOpType.mult,
                )

            # ---- s broadcast to 16 partitions; indicator; recB = 1/s
            psumS = [psS.tile([N, 512], FP, tag="psS", name=f"psS{h}") for h in range(2)]
            ind_sb = small.tile([N, HW], BF, tag="ind")
            for h in range(2):
                nc.tensor.matmul(
                    psumS[h][:], ones16[:], mask_sb[:, 512 * h : 512 * (h + 1)],
                    start=True, stop=True,
                )
                nc.vector.tensor_scalar(
                    ind_sb[:, 512 * h : 512 * (h + 1)], psumS[h][:], kvec[:, 0:1], None,
                    op0=AluOpType.is_equal,
                )

            # ---- mean embedding into the spare 16th column of each eT chunk
            for k in range(2):
                nc.vector.tensor_reduce(
                    eT_sb[k][:, NOBJ : NOBJ + 1], eT_sb[k][:, 0:NOBJ], AX.X, AluOpType.add
                )
                nc.vector.tensor_scalar_mul(
                    eT_sb[k][:, NOBJ : NOBJ + 1], eT_sb[k][:, NOBJ : NOBJ + 1], 1.0 / NOBJ
                )

            # ---- inj = embs_aug @ We -> [16, 256] bf16
            psumI = psI.tile([N, O], FP, tag="psI")
            nc.tensor.matmul(psumI[:], eT_sb[0][:], We_sb[0][:], start=True, stop=False)
            nc.tensor.matmul(psumI[:], eT_sb[1][:], We_sb[1][:], start=False, stop=True)
            inj_sb = small.tile([N, O], BF)
            nc.scalar.activation(inj_sb[:], psumI[:], AF.Copy)

            # ---- main projection matmuls (no stop: injection closes banks)
            psum = [[None, None], [None, None]]
            for oc in range(2):
                for hc in range(2):
                    psum[oc][hc] = psM.tile([128, 512], FP, tag="psM", name=f"psM{oc}{hc}")
            for oc in range(2):
                o0 = 128 * oc
                for h in range(2):
                    for hc in range(2):
                        nc.tensor.matmul(
                            psum[oc][hc][:],
                            Wp_sb[h][:, o0 : o0 + 128],
                            pT_sb[:, HW * h + 512 * hc : HW * h + 512 * (hc + 1)],
                            start=(h == 0), stop=False,
                        )

            # ---- recB via w16 @ ind (banks reused from psS pool)
            recB_sb = small.tile([N, HW], BF, tag="recB")
            psumR = [psS.tile([N, 512], FP, tag="psS", name=f"psR{h}") for h in range(2)]
            for h in range(2):
                nc.tensor.matmul(
                    psumR[h][:], w16[:], ind_sb[:, 512 * h : 512 * (h + 1)],
                    start=True, stop=True,
                )
                nc.scalar.activation(
                    recB_sb[:, 512 * h : 512 * (h + 1)], psumR[h][:], AF.Copy
                )

            # ---- maskN = mask * recB (the /s folded into the mask), halves
            maskN_sb = small.tile([N, HW], BF, tag="maskN")
            for h in range(2):
                nc.vector.tensor_tensor(
                    maskN_sb[:, 512 * h : 512 * (h + 1)],
                    mask_sb[:, 512 * h : 512 * (h + 1)],
                    recB_sb[:, 512 * h : 512 * (h + 1)],
                    op=AluOpType.mult,
                )

            # ---- injection matmuls close each bank; evacuate + store
            o_sb = [outp.tile([128, HW], BF, tag="osb", name=f"osb{oc}") for oc in range(2)]
            for oc in range(2):
                o0 = 128 * oc
                for hc in range(2):
                    nc.tensor.matmul(
                        psum[oc][hc][:],
                        inj_sb[:, o0 : o0 + 128],
                        maskN_sb[:, 512 * hc : 512 * (hc + 1)],
                        start=False, stop=True,
                    )
                    if hc == 0:
                        nc.vector.tensor_copy(
                            o_sb[oc][:, 512 * hc : 512 * (hc + 1)], psum[oc][hc][:]
                        )
                    else:
                        nc.scalar.activation(
                            o_sb[oc][:, 512 * hc : 512 * (hc + 1)], psum[oc][hc][:], AF.Copy
                        )
                eng = nc.sync if oc == 0 else nc.scalar
                eng.dma_start(outT[o0 : o0 + 128, :], o_sb[oc][:])

    nc.compile()
    return nc


def make_in_maps(inputs):
    patches = np.asarray(inputs["patches"], dtype=np.float32)
    embs = np.asarray(inputs["embs"], dtype=np.float32)
    locations = np.asarray(inputs["locations"], dtype=np.int32)
    Wp = np.asarray(inputs["Wp"], dtype=np.float32)
    We = np.asarray(inputs["We"], dtype=np.float32)
    BF_NP = ml_dtypes.bfloat16

    img_box = np.array([[0, 0, H, W]], dtype=np.int32)
    hdr_common = np.zeros((N, HDR), dtype=np.int32)
    hdr_common[:, 4:36] = np.arange(32, dtype=np.int32)[None, :]
    hdr_common[:, 36] = np.arange(1, N + 1, dtype=np.int32)

    wb_common = np.zeros((128, WB), dtype=BF_NP)
    wb_common[:, 0:O] = Wp[0:128].astype(BF_NP)
    wb_common[:, O : 2 * O] = Wp[128:256].astype(BF_NP)
    wb_common[:, 2 * O : 3 * O] = We[0:128].astype(BF_NP)
    wb_common[:, 3 * O : 4 * O] = We[128:256].astype(BF_NP)

    in_maps = []
    for b in range(B):
        hdrb = hdr_common.copy()
        hdrb[:, 0:4] = np.concatenate([locations[b], img_box], 0)
        eTb = embs[b].T.astype(BF_NP)  # [256, 15]
        wbb = wb_common.copy()
        wbb[:, 4 * O : 4 * O + NOBJ] = eTb[0:128]
        wbb[:, 4 * O + N : 4 * O + N + NOBJ] = eTb[128:256]
        pTb = patches[b].reshape(HW, D).T.astype(BF_NP)  # [256, 1024]
        pT2 = np.concatenate([pTb[0:128], pTb[128:256]], axis=1)  # [128, 2048]
        in_maps.append(
            {
                "hdr": np.ascontiguousarray(hdrb),
                "wb": wbb,
                "pT": np.ascontiguousarray(pT2),
            }
        )
    return in_maps


_NC = None


def _get_nc():
    global _NC
    if _NC is None:
        _NC = build_nc(debug=False)
    return _NC


def run(inputs, trace: bool = False, **kwargs):
    nc = _get_nc()
    res = bass_utils.run_bass_kernel_spmd(
        nc, make_in_maps(inputs), core_ids=list(range(B)), trace=trace, **kwargs
    )
    full = np.stack(
        [res.results[b]["outT"].astype(np.float32).T for b in range(B)], axis=0
    )
    return np.ascontiguousarray(full), res


def kernel(**inputs) -> np.ndarray:
    full, _ = run(inputs, trace=False)
    return full
